# revision 9
# baseline (speedup 1.0000x reference)
"""Trainium2 Bass kernel for DiffusionConv (Chebyshev graph diffusion).

Math (reference):
    x0 = [feat; feat]                       # [2N, T*F]
    x1 = A @ x0                             # A sparse: A[dst, src] = sum ef
    x2 = 2*A@x1 - x0 ; x3 = 2*A@x2 - x1
    out = concat([feat, x1[:N], x1[N:], x2[:N], x2[N:], x3[:N], x3[N:]]) @ W + b

Strategy (8 NeuronCores, SPMD single program):
  - Edges sorted by dst; dst-range sharded 8-way (6250 slots/core).
  - Per core, edges packed into tiles of <=128 distinct dst slots; within a
    tile edges are split by source half (fwd: src<N, bwd: src>=N) so the Q7
    dma_gather op can use int16 row indices into per-half tables.
  - Per (tile, half): one dma_gather pulls up to CPF*128 source rows (each
    T*F values) into SBUF; a one-hot scatter matrix S (built on the vector
    engine via iota==dstslot fused with *ef) folds the per-edge multiply and
    the segment-sum into TensorE matmuls accumulating in PSUM.
  - Chebyshev combine on DVE (the x_{k-2} operand is the step-(k-2) result
    tile kept in SBUF; for k=2 it is a per-tile indirect gather from feat),
    results scattered into a per-core slice, then an AllGather replicates
    the new diffusion state for the next step (x3: pairwise exchange only).
  - Final linear: indirect-gather block rows, transpose on TensorE,
    out^T = W^T @ hcat^T accumulated in PSUM; host reassembles.

The full (unsharded) inputs come in; host-side numpy does index/layout
preprocessing only (sorting, tiling, padding) - all FLOPs of the module
run on the NeuronCores.
"""

import os
import sys

sys.path.insert(0, "/opt/trn_rl_repo")

import numpy as np

import concourse.bacc as bacc
import concourse.bass as bass
import concourse.mybir as mybir
import concourse.tile as tile

# ---------------------------------------------------------------- problem dims
N = int(os.environ.get("DIFF_N", 25000))
T = 8
F = 32
OUTF = 64
STEPS = 3
ROW = T * F            # 256 values per node-row
TWO_N = 2 * N
CORES = 8
SPC = TWO_N // CORES   # dst slots per core
CPF = int(os.environ.get("DIFF_CPF", 8))  # chunks (of 128 edges) per tile half
# NOTE: dma_gather faults above 1024 indices per op, so CPF*128 <= 1024.
CAP = CPF * 128        # edge capacity per tile half
NFIN = N // CORES      # final-linear rows (n) per core
NFCH = (NFIN + 127) // 128  # final-linear chunks

FP32 = mybir.dt.float32
BF16 = mybir.dt.bfloat16
I32 = mybir.dt.int32
I16 = mybir.dt.int16

# table / gather dtype: float32 (precise) or bfloat16 (half the DMA traffic)
TAB_DT = BF16 if os.environ.get("DIFF_TAB_BF16", "1") == "1" else FP32
TAB_NP = np.dtype("bfloat16") if TAB_DT is BF16 else np.dtype("float32")


# ================================================================ host prep
def _prep(feat, ef, W, b, src, dst):
    """Build per-core tiled edge metadata. Returns (in_maps, ntiles)."""
    feat = np.ascontiguousarray(np.asarray(feat), dtype=np.float32).reshape(N, ROW)
    ef = np.asarray(ef, dtype=np.float32)
    src = np.asarray(src, dtype=np.int64)
    dst = np.asarray(dst, dtype=np.int64)

    order = np.argsort(dst, kind="stable")
    s_src = src[order]
    s_dst = dst[order]
    s_ef = ef[order]

    core_edge_bounds = np.searchsorted(s_dst, np.arange(CORES + 1) * SPC)

    # ---- per-core greedy tiling (capacity per source half)
    per_core = []
    for c in range(CORES):
        lo, hi = core_edge_bounds[c], core_edge_bounds[c + 1]
        cs, cd, ce = s_src[lo:hi], s_dst[lo:hi] - c * SPC, s_ef[lo:hi]
        fwd_mask = cs < N
        counts_f = np.bincount(cd[fwd_mask], minlength=SPC)
        counts_b = np.bincount(cd[~fwd_mask], minlength=SPC)
        starts = np.concatenate([[0], np.cumsum(counts_f + counts_b)])
        tiles = []  # (slot_lo, slot_hi, edge_lo, edge_hi)
        slot = 0
        while slot < SPC:
            t_lo = slot
            nf = nb = 0
            while (
                slot < SPC
                and slot - t_lo < 128
                and nf + counts_f[slot] <= CAP
                and nb + counts_b[slot] <= CAP
            ):
                nf += counts_f[slot]
                nb += counts_b[slot]
                slot += 1
            if slot == t_lo:
                raise ValueError("node degree exceeds tile capacity")
            tiles.append((t_lo, slot, starts[t_lo], starts[slot]))
        per_core.append((cs, cd, ce, tiles))

    ntiles = max(len(p[3]) for p in per_core)

    in_maps = []
    for c in range(CORES):
        cs, cd, ce, tiles = per_core[c]
        gidx = np.zeros((128, ntiles, 2, CAP // 16), np.int16)
        sef = np.zeros((128, ntiles, 2, 2 * CPF), np.float32)
        pix = np.zeros((128, 2, ntiles), np.int32)
        pix[:, 0, :] = SPC  # null tiles / pad slots scatter to trash row
        for t, (sl, sh, el, eh) in enumerate(tiles):
            e_src = cs[el:eh]
            e_slot = cd[el:eh] - sl
            e_w = ce[el:eh]
            for side in (0, 1):
                m = (e_src < N) if side == 0 else (e_src >= N)
                s_idx = (e_src[m] - side * N).astype(np.int64)
                s_slot = e_slot[m]
                s_w = e_w[m]
                pad = CAP - len(s_idx)
                s_idx = np.concatenate([s_idx, np.zeros(pad, np.int64)])
                s_slot = np.concatenate([s_slot, np.zeros(pad, np.int64)])
                s_w = np.concatenate([s_w, np.zeros(pad, np.float32)])
                # edge g -> (partition g%128, chunk g//128); idx wrapped by 16
                gidx[:, t, side, :] = np.tile(
                    s_idx.astype(np.int16).reshape(-1, 16).T, (8, 1)
                )
                sef[:, t, 0, side * CPF : (side + 1) * CPF] = s_slot.reshape(
                    CPF, 128
                ).T
                sef[:, t, 1, side * CPF : (side + 1) * CPF] = s_w.reshape(
                    CPF, 128
                ).T
            nslots = sh - sl
            p = np.arange(128)
            glob = c * SPC + sl + p
            valid = p < nslots
            pix[:, 0, t] = np.where(valid, sl + p, SPC)        # slice row
            pix[:, 1, t] = np.where(valid, glob % N, 0)        # k=1 xp (feat row)

        # final-linear row indices
        if c < 4:
            nbase = c * SPC
        else:
            nbase = (c - 4) * SPC + NFIN
        pairbase = (c % 4) * SPC
        lin = np.zeros((128, 2, NFCH), np.int32)
        for ch in range(NFCH):
            p = np.arange(128)
            nl = ch * 128 + p
            ng = nbase + np.minimum(nl, NFIN - 1)
            lin[:, 0, ch] = ng
            lin[:, 1, ch] = ng - pairbase

        in_maps.append(
            {
                "feat": feat.astype(TAB_NP, copy=True),
                "gidx": gidx.reshape(128, ntiles * 2 * (CAP // 16)).copy(),
                "sef": sef.reshape(128, ntiles * 4 * CPF).copy(),
                "pix": pix.reshape(128, 2 * ntiles).copy(),
                "lin": lin.reshape(128, 2 * NFCH).copy(),
                "wmat": np.asarray(W, np.float32).astype(np.dtype("bfloat16")),
                "bvec": np.asarray(b, np.float32).reshape(OUTF, 1).copy(),
                "iota": np.broadcast_to(
                    np.arange(128, dtype=np.float32).astype(TAB_NP), (128, 128)
                ).copy(),
                "ident": np.eye(128, dtype=np.dtype("bfloat16")),
            }
        )
    return in_maps, ntiles


# ================================================================ bass program
def _build(ntiles):
    nc = bacc.Bacc(
        "TRN2", target_bir_lowering=False, debug=False, num_devices=CORES
    )

    feat_t = nc.dram_tensor("feat", [N, ROW], TAB_DT, kind="ExternalInput")
    gidx_t = nc.dram_tensor(
        "gidx", [128, ntiles * 2 * (CAP // 16)], I16, kind="ExternalInput"
    )
    sef_t = nc.dram_tensor(
        "sef", [128, ntiles * 4 * CPF], FP32, kind="ExternalInput"
    )
    pix_t = nc.dram_tensor("pix", [128, 2 * ntiles], I32, kind="ExternalInput")
    lin_t = nc.dram_tensor("lin", [128, 2 * NFCH], I32, kind="ExternalInput")
    w_t = nc.dram_tensor("wmat", [7 * F, OUTF], BF16, kind="ExternalInput")
    b_t = nc.dram_tensor("bvec", [OUTF, 1], FP32, kind="ExternalInput")
    iota_t = nc.dram_tensor("iota", [128, 128], TAB_DT, kind="ExternalInput")
    ident_t = nc.dram_tensor("ident", [128, 128], BF16, kind="ExternalInput")

    outT = nc.dram_tensor("outT", [T, OUTF, NFIN], FP32, kind="ExternalOutput")

    # internal DRAM
    slice_bufs = [
        nc.dram_tensor(f"slice{k}", [SPC + 1, ROW], TAB_DT) for k in range(STEPS)
    ]
    x1t = nc.dram_tensor("x1t", [TWO_N, ROW], TAB_DT, addr_space="Shared")
    x2t = nc.dram_tensor("x2t", [TWO_N, ROW], TAB_DT, addr_space="Shared")
    x3t = nc.dram_tensor("x3t", [2 * SPC, ROW], TAB_DT)

    rg_all = [list(range(CORES))]
    rg_pair = [[c, c + 4] for c in range(4)]

    ICW = CAP // 16  # idx columns per (tile, half)

    with tile.TileContext(nc, num_cores=CORES) as tc:
        with (
            tc.tile_pool(name="const", bufs=1) as constp,
            tc.tile_pool(name="meta", bufs=1) as metap,
        ):
            iota_s = constp.tile([128, 128], TAB_DT)
            nc.sync.dma_start(iota_s[:], iota_t[:])
            ident_s = constp.tile([128, 128], BF16)
            nc.sync.dma_start(ident_s[:], ident_t[:])
            wa_s = constp.tile([128, OUTF], BF16)
            nc.sync.dma_start(wa_s[:], w_t[0:128, :])
            wb_s = constp.tile([96, OUTF], BF16)
            nc.sync.dma_start(wb_s[:], w_t[128 : 7 * F, :])
            bias_s = constp.tile([OUTF, 1], FP32)
            nc.sync.dma_start(bias_s[:], b_t[:])

            gidx_s = metap.tile([128, ntiles * 2 * ICW], I16)
            nc.sync.dma_start(gidx_s[:], gidx_t[:])
            sef_s = metap.tile([128, ntiles * 4 * CPF], FP32)
            nc.sync.dma_start(sef_s[:], sef_t[:])
            pix_s = metap.tile([128, 2 * ntiles], I32)
            nc.sync.dma_start(pix_s[:], pix_t[:])
            lin_s = metap.tile([128, 2 * NFCH], I32)
            nc.sync.dma_start(lin_s[:], lin_t[:])

            # ---------------- diffusion steps
            with (
                tc.tile_pool(name="big", bufs=1) as bigp,
                tc.tile_pool(name="gat", bufs=2) as gatp,
                tc.tile_pool(name="sml", bufs=3) as smlp,
                tc.tile_pool(name="xpp", bufs=2) as xpp,
                tc.tile_pool(name="ps", bufs=2, space="PSUM") as psp,
            ):
                y_bufs = []
                for k in range(STEPS):
                    if k == 0:
                        halves = [feat_t[0:N, :], feat_t[0:N, :]]
                    elif k == 1:
                        halves = [x1t[0:N, :], x1t[N:TWO_N, :]]
                    else:
                        halves = [x2t[0:N, :], x2t[N:TWO_N, :]]
                    y_all = bigp.tile(
                        [128, ntiles * ROW], TAB_DT, tag=f"y{k}", name=f"y{k}"
                    )
                    y_bufs.append(y_all)
                    for t in range(ntiles):
                        psum = psp.tile([128, ROW], FP32, tag="acc")
                        for side in (0, 1):
                            G = gatp.tile(
                                [128, CPF * ROW], TAB_DT, tag=f"G{side}",
                                name=f"G{side}",
                            )
                            icb = (t * 2 + side) * ICW
                            nc.gpsimd.dma_gather(
                                G[:].rearrange("p (c r) -> p c r", c=CPF),
                                halves[side],
                                gidx_s[:, icb : icb + ICW],
                                CAP,
                                CAP,
                                ROW,
                            )
                            for j in range(CPF):
                                cc = side * CPF + j
                                dcol = (t * 2 + 0) * 2 * CPF + cc
                                ecol = (t * 2 + 1) * 2 * CPF + cc
                                S = smlp.tile([128, 128], TAB_DT, tag="S")
                                nc.vector.tensor_scalar(
                                    out=S[:],
                                    in0=iota_s[:],
                                    scalar1=sef_s[:, dcol : dcol + 1],
                                    scalar2=sef_s[:, ecol : ecol + 1],
                                    op0=mybir.AluOpType.is_equal,
                                    op1=mybir.AluOpType.mult,
                                )
                                nc.tensor.matmul(
                                    out=psum[:],
                                    lhsT=S[:],
                                    rhs=G[:, j * ROW : (j + 1) * ROW],
                                    start=(side == 0 and j == 0),
                                    stop=(side == 1 and j == CPF - 1),
                                )
                        y_sl = y_all[:, t * ROW : (t + 1) * ROW]
                        if k == 0:
                            nc.vector.tensor_copy(y_sl, psum[:])
                        else:
                            if k == 1:
                                xp = xpp.tile([128, ROW], TAB_DT, tag="xp")
                                nc.gpsimd.indirect_dma_start(
                                    out=xp[:],
                                    out_offset=None,
                                    in_=feat_t[:],
                                    in_offset=bass.IndirectOffsetOnAxis(
                                        ap=pix_s[:, ntiles + t : ntiles + t + 1],
                                        axis=0,
                                    ),
                                )
                                xp_sl = xp[:]
                            else:
                                xp_sl = y_bufs[0][:, t * ROW : (t + 1) * ROW]
                            nc.vector.tensor_scalar(
                                out=y_sl,
                                in0=psum[:],
                                scalar1=2.0,
                                scalar2=None,
                                op0=mybir.AluOpType.mult,
                            )
                            nc.vector.tensor_tensor(
                                out=y_sl,
                                in0=y_sl,
                                in1=xp_sl,
                                op=mybir.AluOpType.subtract,
                            )
                        # scatter this tile's rows into the per-core slice
                        nc.gpsimd.indirect_dma_start(
                            out=slice_bufs[k][:],
                            out_offset=bass.IndirectOffsetOnAxis(
                                ap=pix_s[:, t : t + 1], axis=0
                            ),
                            in_=y_sl,
                            in_offset=None,
                        )
                    nc.gpsimd.collective_compute(
                        "AllGather",
                        mybir.AluOpType.bypass,
                        replica_groups=rg_all if k < STEPS - 1 else rg_pair,
                        ins=[slice_bufs[k][0:SPC, :].opt()],
                        outs=[(x1t, x2t, x3t)[k][:].opt()],
                    )

            # ---------------- final linear
            blocks = [
                (feat_t, 0, 0),
                (x1t, 0, 0),
                (x1t, N * ROW, 0),
                (x2t, 0, 0),
                (x2t, N * ROW, 0),
                (x3t, 0, 1),
                (x3t, SPC * ROW, 1),
            ]
            with (
                tc.tile_pool(name="fin", bufs=2) as finp,
                tc.tile_pool(name="fps", bufs=2, space="PSUM") as fpsp,
                tc.tile_pool(name="fps2", bufs=2, space="PSUM") as fpsp2,
            ):
                for ch in range(NFCH):
                    nrows = min(128, NFIN - ch * 128)
                    hA = [
                        finp.tile([128, 128], BF16, tag=f"hA{tt}", name=f"hA{tt}")
                        for tt in range(T)
                    ]
                    hB = [
                        finp.tile([96, 128], BF16, tag=f"hB{tt}", name=f"hB{tt}")
                        for tt in range(T)
                    ]
                    for bi, (tab, eoff, licol) in enumerate(blocks):
                        Gf = finp.tile([128, ROW], BF16, tag="Gf")
                        nc.gpsimd.indirect_dma_start(
                            out=Gf[:],
                            out_offset=None,
                            in_=tab[:],
                            in_offset=bass.IndirectOffsetOnAxis(
                                ap=lin_s[:, licol * NFCH + ch : licol * NFCH + ch + 1],
                                axis=0,
                            ),
                            element_offset=eoff,
                        )
                        for hh in range(2):  # halves of the (t,f) axis
                            ptr = fpsp.tile([128, 128], BF16, tag="ptr")
                            nc.tensor.transpose(
                                out=ptr[:],
                                in_=Gf[:, hh * 128 : (hh + 1) * 128],
                                identity=ident_s[:],
                            )
                            for q in range(4):  # t within this half
                                tt = hh * 4 + q
                                dst, off = (
                                    (hA[tt], bi * F)
                                    if bi < 4
                                    else (hB[tt], (bi - 4) * F)
                                )
                                nc.vector.tensor_copy(
                                    dst[off : off + F, :],
                                    ptr[q * F : (q + 1) * F, :],
                                )
                    stage = finp.tile([OUTF, T * 128], FP32, tag="stage")
                    for tt in range(T):
                        ps2 = fpsp2.tile([OUTF, 128], FP32, tag="ps2")
                        nc.tensor.matmul(
                            out=ps2[:], lhsT=wa_s[:], rhs=hA[tt][:],
                            start=True, stop=False,
                        )
                        nc.tensor.matmul(
                            out=ps2[:], lhsT=wb_s[:], rhs=hB[tt][:],
                            start=False, stop=True,
                        )
                        nc.vector.tensor_scalar(
                            out=stage[:, tt * 128 : tt * 128 + 128],
                            in0=ps2[:],
                            scalar1=bias_s[:],
                            scalar2=None,
                            op0=mybir.AluOpType.add,
                        )
                    # outT[t, o, ch*128 : ch*128+nrows] = stage[o, t*128 : ...]
                    st = stage[:]
                    out_ap = bass.AP(
                        outT,
                        ch * 128,
                        [[NFIN, OUTF], [OUTF * NFIN, T], [1, nrows]],
                    )
                    st_ap = bass.AP(
                        st.tensor, st.offset, [st.ap[0], [128, T], [1, nrows]]
                    )
                    nc.sync.dma_start(out=out_ap, in_=st_ap)

    nc.compile()
    return nc


_NC_CACHE = {}


def _get_nc(ntiles):
    if ntiles not in _NC_CACHE:
        _NC_CACHE[ntiles] = _build(ntiles)
    return _NC_CACHE[ntiles]


# ================================================================ entry point
def kernel(feat, ef, W, b, src, dst):
    from concourse.bass_utils import run_bass_kernel_spmd

    in_maps, ntiles = _prep(feat, ef, W, b, src, dst)
    nc = _get_nc(ntiles)
    res = run_bass_kernel_spmd(nc, in_maps, core_ids=list(range(CORES)))
    out = np.zeros((N, T, OUTF), np.float32)
    for c in range(CORES):
        o = np.asarray(res.results[c]["outT"], np.float32).reshape(T, OUTF, NFIN)
        if c < 4:
            nbase = c * SPC
        else:
            nbase = (c - 4) * SPC + NFIN
        out[nbase : nbase + NFIN] = o.transpose(2, 0, 1)
    return out


# revision 11
# speedup vs baseline: 1.0833x; 1.0833x over previous
"""Trainium2 Bass kernel for DiffusionConv (Chebyshev graph diffusion).

Math (reference):
    x0 = [feat; feat]                       # [2N, T*F]
    x1 = A @ x0                             # A sparse: A[dst, src] = sum ef
    x2 = 2*A@x1 - x0 ; x3 = 2*A@x2 - x1
    out = concat([feat, x1[:N], x1[N:], x2[:N], x2[N:], x3[:N], x3[N:]]) @ W + b

Strategy (8 NeuronCores, SPMD single program):
  - Edges sorted by dst; dst-range sharded 8-way (6250 slots/core).
  - Per core, edges packed into tiles of <=128 distinct dst slots; within a
    tile edges are split by source half (fwd: src<N, bwd: src>=N) so the Q7
    dma_gather op can use int16 row indices into per-half tables.
  - Per (tile, half): one dma_gather pulls up to CPF*128 source rows (each
    T*F values) into SBUF; a one-hot scatter matrix S (built on the vector
    engine via iota==dstslot fused with *ef) folds the per-edge multiply and
    the segment-sum into TensorE matmuls accumulating in PSUM.
  - Chebyshev combine on DVE (the x_{k-2} operand is the step-(k-2) result
    tile kept in SBUF; for k=2 it is a per-tile indirect gather from feat),
    results scattered into a per-core slice, then an AllGather replicates
    the new diffusion state for the next step (x3: pairwise exchange only).
  - Final linear: indirect-gather block rows, transpose on TensorE,
    out^T = W^T @ hcat^T accumulated in PSUM; host reassembles.

The full (unsharded) inputs come in; host-side numpy does index/layout
preprocessing only (sorting, tiling, padding) - all FLOPs of the module
run on the NeuronCores.
"""

import os
import sys

sys.path.insert(0, "/opt/trn_rl_repo")

import numpy as np

import concourse.bacc as bacc
import concourse.bass as bass
import concourse.mybir as mybir
import concourse.tile as tile

# ---------------------------------------------------------------- problem dims
N = int(os.environ.get("DIFF_N", 25000))
T = 8
F = 32
OUTF = 64
STEPS = 3
ROW = T * F            # 256 values per node-row
TWO_N = 2 * N
CORES = 8
SPC = TWO_N // CORES   # dst slots per core
CPF = int(os.environ.get("DIFF_CPF", 8))  # chunks (of 128 edges) per tile half
# NOTE: dma_gather faults above 1024 indices per op, so CPF*128 <= 1024.
CAP = CPF * 128        # edge capacity per tile half
NFIN = N // CORES      # final-linear rows (n) per core
NFCH = (NFIN + 127) // 128  # final-linear chunks

FP32 = mybir.dt.float32
BF16 = mybir.dt.bfloat16
I32 = mybir.dt.int32
I16 = mybir.dt.int16

# table / gather dtype: float32 (precise) or bfloat16 (half the DMA traffic)
TAB_DT = BF16 if os.environ.get("DIFF_TAB_BF16", "1") == "1" else FP32
TAB_NP = np.dtype("bfloat16") if TAB_DT is BF16 else np.dtype("float32")


# ================================================================ host prep
def _prep(feat, ef, W, b, src, dst):
    """Build per-core tiled edge metadata. Returns (in_maps, ntiles)."""
    feat = np.ascontiguousarray(np.asarray(feat), dtype=np.float32).reshape(N, ROW)
    ef = np.asarray(ef, dtype=np.float32)
    src = np.asarray(src, dtype=np.int64)
    dst = np.asarray(dst, dtype=np.int64)

    order = np.argsort(dst, kind="stable")
    s_src = src[order]
    s_dst = dst[order]
    s_ef = ef[order]

    core_edge_bounds = np.searchsorted(s_dst, np.arange(CORES + 1) * SPC)

    # ---- per-core greedy tiling (capacity per source half)
    per_core = []
    for c in range(CORES):
        lo, hi = core_edge_bounds[c], core_edge_bounds[c + 1]
        cs, cd, ce = s_src[lo:hi], s_dst[lo:hi] - c * SPC, s_ef[lo:hi]
        fwd_mask = cs < N
        counts_f = np.bincount(cd[fwd_mask], minlength=SPC)
        counts_b = np.bincount(cd[~fwd_mask], minlength=SPC)
        starts = np.concatenate([[0], np.cumsum(counts_f + counts_b)])
        tiles = []  # (slot_lo, slot_hi, edge_lo, edge_hi)
        slot = 0
        while slot < SPC:
            t_lo = slot
            nf = nb = 0
            while (
                slot < SPC
                and slot - t_lo < 128
                and nf + counts_f[slot] <= CAP
                and nb + counts_b[slot] <= CAP
            ):
                nf += counts_f[slot]
                nb += counts_b[slot]
                slot += 1
            if slot == t_lo:
                raise ValueError("node degree exceeds tile capacity")
            tiles.append((t_lo, slot, starts[t_lo], starts[slot]))
        per_core.append((cs, cd, ce, tiles))

    ntiles = max(len(p[3]) for p in per_core)

    in_maps = []
    for c in range(CORES):
        cs, cd, ce, tiles = per_core[c]
        gidx = np.zeros((128, ntiles, 2, CAP // 16), np.int16)
        smat = np.zeros((ntiles, 128, 2 * CPF, 128), np.float32)
        pix = np.zeros((128, 2, ntiles), np.int32)
        pix[:, 0, :] = SPC  # null tiles / pad slots scatter to trash row
        for t, (sl, sh, el, eh) in enumerate(tiles):
            e_src = cs[el:eh]
            e_slot = cd[el:eh] - sl
            e_w = ce[el:eh]
            for side in (0, 1):
                m = (e_src < N) if side == 0 else (e_src >= N)
                s_idx = (e_src[m] - side * N).astype(np.int64)
                s_slot = e_slot[m]
                s_w = e_w[m]
                pad = CAP - len(s_idx)
                s_idx = np.concatenate([s_idx, np.zeros(pad, np.int64)])
                s_slot = np.concatenate([s_slot, np.zeros(pad, np.int64)])
                s_w = np.concatenate([s_w, np.zeros(pad, np.float32)])
                # edge g -> (partition g%128, chunk g//128); idx wrapped by 16
                gidx[:, t, side, :] = np.tile(
                    s_idx.astype(np.int16).reshape(-1, 16).T, (8, 1)
                )
                # S[p, cc, slot] = ef for chunk cc = side*CPF + g//128
                e_p = np.arange(CAP) % 128
                e_cc = side * CPF + np.arange(CAP) // 128
                smat[t, e_p, e_cc, s_slot] += s_w
            nslots = sh - sl
            p = np.arange(128)
            glob = c * SPC + sl + p
            valid = p < nslots
            pix[:, 0, t] = np.where(valid, sl + p, SPC)        # slice row
            pix[:, 1, t] = np.where(valid, glob % N, 0)        # k=1 xp (feat row)

        # final-linear row indices
        if c < 4:
            nbase = c * SPC
        else:
            nbase = (c - 4) * SPC + NFIN
        pairbase = (c % 4) * SPC
        lin = np.zeros((128, 2, NFCH), np.int32)
        for ch in range(NFCH):
            p = np.arange(128)
            nl = ch * 128 + p
            ng = nbase + np.minimum(nl, NFIN - 1)
            lin[:, 0, ch] = ng
            lin[:, 1, ch] = ng - pairbase

        in_maps.append(
            {
                "feat": feat.astype(TAB_NP, copy=True),
                "gidx": gidx.reshape(128, ntiles * 2 * (CAP // 16)).copy(),
                "smat": smat.reshape(ntiles * 128, 2 * CPF * 128).astype(TAB_NP),
                "pix": pix.reshape(128, 2 * ntiles).copy(),
                "lin": lin.reshape(128, 2 * NFCH).copy(),
                "wmat": np.asarray(W, np.float32).astype(np.dtype("bfloat16")),
                "bvec": np.asarray(b, np.float32).reshape(OUTF, 1).copy(),
                "ident": np.eye(128, dtype=np.dtype("bfloat16")),
            }
        )
    return in_maps, ntiles


# ================================================================ bass program
def _build(ntiles):
    nc = bacc.Bacc(
        "TRN2", target_bir_lowering=False, debug=False, num_devices=CORES
    )

    feat_t = nc.dram_tensor("feat", [N, ROW], TAB_DT, kind="ExternalInput")
    gidx_t = nc.dram_tensor(
        "gidx", [128, ntiles * 2 * (CAP // 16)], I16, kind="ExternalInput"
    )
    smat_t = nc.dram_tensor(
        "smat", [ntiles * 128, 2 * CPF * 128], TAB_DT, kind="ExternalInput"
    )
    pix_t = nc.dram_tensor("pix", [128, 2 * ntiles], I32, kind="ExternalInput")
    lin_t = nc.dram_tensor("lin", [128, 2 * NFCH], I32, kind="ExternalInput")
    w_t = nc.dram_tensor("wmat", [7 * F, OUTF], BF16, kind="ExternalInput")
    b_t = nc.dram_tensor("bvec", [OUTF, 1], FP32, kind="ExternalInput")
    ident_t = nc.dram_tensor("ident", [128, 128], BF16, kind="ExternalInput")

    outT = nc.dram_tensor("outT", [T, OUTF, NFIN], FP32, kind="ExternalOutput")

    # internal DRAM
    slice_bufs = [
        nc.dram_tensor(f"slice{k}", [SPC + 1, ROW], TAB_DT) for k in range(STEPS)
    ]
    x1t = nc.dram_tensor("x1t", [TWO_N, ROW], TAB_DT, addr_space="Shared")
    x2t = nc.dram_tensor("x2t", [TWO_N, ROW], TAB_DT, addr_space="Shared")
    x3t = nc.dram_tensor("x3t", [2 * SPC, ROW], TAB_DT)

    rg_all = [list(range(CORES))]
    rg_pair = [[c, c + 4] for c in range(4)]

    ICW = CAP // 16  # idx columns per (tile, half)

    with tile.TileContext(nc, num_cores=CORES) as tc:
        with (
            tc.tile_pool(name="const", bufs=1) as constp,
            tc.tile_pool(name="meta", bufs=1) as metap,
        ):
            ident_s = constp.tile([128, 128], BF16)
            nc.sync.dma_start(ident_s[:], ident_t[:])
            wa_s = constp.tile([128, OUTF], BF16)
            nc.sync.dma_start(wa_s[:], w_t[0:128, :])
            wb_s = constp.tile([96, OUTF], BF16)
            nc.sync.dma_start(wb_s[:], w_t[128 : 7 * F, :])
            bias_s = constp.tile([OUTF, 1], FP32)
            nc.sync.dma_start(bias_s[:], b_t[:])

            gidx_s = metap.tile([128, ntiles * 2 * ICW], I16)
            nc.sync.dma_start(gidx_s[:], gidx_t[:])
            pix_s = metap.tile([128, 2 * ntiles], I32)
            nc.sync.dma_start(pix_s[:], pix_t[:])
            lin_s = metap.tile([128, 2 * NFCH], I32)
            nc.sync.dma_start(lin_s[:], lin_t[:])

            # ---------------- diffusion steps
            with (
                tc.tile_pool(name="big", bufs=1) as bigp,
                tc.tile_pool(name="gat", bufs=2) as gatp,
                tc.tile_pool(name="sml", bufs=3) as smlp,
                tc.tile_pool(name="xpp", bufs=2) as xpp,
                tc.tile_pool(name="ps", bufs=2, space="PSUM") as psp,
            ):
                y_bufs = []
                for k in range(STEPS):
                    if k == 0:
                        halves = [feat_t[0:N, :], feat_t[0:N, :]]
                    elif k == 1:
                        halves = [x1t[0:N, :], x1t[N:TWO_N, :]]
                    else:
                        halves = [x2t[0:N, :], x2t[N:TWO_N, :]]
                    y_all = bigp.tile(
                        [128, ntiles * ROW], TAB_DT, tag=f"y{k}", name=f"y{k}"
                    )
                    y_bufs.append(y_all)
                    for t in range(ntiles):
                        psum = psp.tile([128, ROW], FP32, tag="acc")
                        Ssb = smlp.tile([128, 2 * CPF * 128], TAB_DT, tag="S")
                        nc.sync.dma_start(Ssb[:], smat_t[t * 128 : (t + 1) * 128, :])
                        for side in (0, 1):
                            G = gatp.tile(
                                [128, CPF * ROW], TAB_DT, tag=f"G{side}",
                                name=f"G{side}",
                            )
                            icb = (t * 2 + side) * ICW
                            nc.gpsimd.dma_gather(
                                G[:].rearrange("p (c r) -> p c r", c=CPF),
                                halves[side],
                                gidx_s[:, icb : icb + ICW],
                                CAP,
                                CAP,
                                ROW,
                            )
                            for j in range(CPF):
                                cc = side * CPF + j
                                nc.tensor.matmul(
                                    out=psum[:],
                                    lhsT=Ssb[:, cc * 128 : (cc + 1) * 128],
                                    rhs=G[:, j * ROW : (j + 1) * ROW],
                                    start=(side == 0 and j == 0),
                                    stop=(side == 1 and j == CPF - 1),
                                )
                        y_sl = y_all[:, t * ROW : (t + 1) * ROW]
                        if k == 0:
                            nc.vector.tensor_copy(y_sl, psum[:])
                        else:
                            if k == 1:
                                xp = xpp.tile([128, ROW], TAB_DT, tag="xp")
                                nc.gpsimd.indirect_dma_start(
                                    out=xp[:],
                                    out_offset=None,
                                    in_=feat_t[:],
                                    in_offset=bass.IndirectOffsetOnAxis(
                                        ap=pix_s[:, ntiles + t : ntiles + t + 1],
                                        axis=0,
                                    ),
                                )
                                xp_sl = xp[:]
                            else:
                                xp_sl = y_bufs[0][:, t * ROW : (t + 1) * ROW]
                            nc.vector.tensor_scalar(
                                out=y_sl,
                                in0=psum[:],
                                scalar1=2.0,
                                scalar2=None,
                                op0=mybir.AluOpType.mult,
                            )
                            nc.vector.tensor_tensor(
                                out=y_sl,
                                in0=y_sl,
                                in1=xp_sl,
                                op=mybir.AluOpType.subtract,
                            )
                        # scatter this tile's rows into the per-core slice
                        nc.gpsimd.indirect_dma_start(
                            out=slice_bufs[k][:],
                            out_offset=bass.IndirectOffsetOnAxis(
                                ap=pix_s[:, t : t + 1], axis=0
                            ),
                            in_=y_sl,
                            in_offset=None,
                        )
                    nc.gpsimd.collective_compute(
                        "AllGather",
                        mybir.AluOpType.bypass,
                        replica_groups=rg_all if k < STEPS - 1 else rg_pair,
                        ins=[slice_bufs[k][0:SPC, :].opt()],
                        outs=[(x1t, x2t, x3t)[k][:].opt()],
                    )

            # ---------------- final linear
            blocks = [
                (feat_t, 0, 0),
                (x1t, 0, 0),
                (x1t, N * ROW, 0),
                (x2t, 0, 0),
                (x2t, N * ROW, 0),
                (x3t, 0, 1),
                (x3t, SPC * ROW, 1),
            ]
            with (
                tc.tile_pool(name="fin", bufs=2) as finp,
                tc.tile_pool(name="fps", bufs=2, space="PSUM") as fpsp,
                tc.tile_pool(name="fps2", bufs=2, space="PSUM") as fpsp2,
            ):
                for ch in range(NFCH):
                    nrows = min(128, NFIN - ch * 128)
                    hA = [
                        finp.tile([128, 128], BF16, tag=f"hA{tt}", name=f"hA{tt}")
                        for tt in range(T)
                    ]
                    hB = [
                        finp.tile([96, 128], BF16, tag=f"hB{tt}", name=f"hB{tt}")
                        for tt in range(T)
                    ]
                    for bi, (tab, eoff, licol) in enumerate(blocks):
                        Gf = finp.tile([128, ROW], BF16, tag="Gf")
                        nc.gpsimd.indirect_dma_start(
                            out=Gf[:],
                            out_offset=None,
                            in_=tab[:],
                            in_offset=bass.IndirectOffsetOnAxis(
                                ap=lin_s[:, licol * NFCH + ch : licol * NFCH + ch + 1],
                                axis=0,
                            ),
                            element_offset=eoff,
                        )
                        for hh in range(2):  # halves of the (t,f) axis
                            ptr = fpsp.tile([128, 128], BF16, tag="ptr")
                            nc.tensor.transpose(
                                out=ptr[:],
                                in_=Gf[:, hh * 128 : (hh + 1) * 128],
                                identity=ident_s[:],
                            )
                            for q in range(4):  # t within this half
                                tt = hh * 4 + q
                                dst, off = (
                                    (hA[tt], bi * F)
                                    if bi < 4
                                    else (hB[tt], (bi - 4) * F)
                                )
                                nc.vector.tensor_copy(
                                    dst[off : off + F, :],
                                    ptr[q * F : (q + 1) * F, :],
                                )
                    stage = finp.tile([OUTF, T * 128], FP32, tag="stage")
                    for tt in range(T):
                        ps2 = fpsp2.tile([OUTF, 128], FP32, tag="ps2")
                        nc.tensor.matmul(
                            out=ps2[:], lhsT=wa_s[:], rhs=hA[tt][:],
                            start=True, stop=False,
                        )
                        nc.tensor.matmul(
                            out=ps2[:], lhsT=wb_s[:], rhs=hB[tt][:],
                            start=False, stop=True,
                        )
                        nc.vector.tensor_scalar(
                            out=stage[:, tt * 128 : tt * 128 + 128],
                            in0=ps2[:],
                            scalar1=bias_s[:],
                            scalar2=None,
                            op0=mybir.AluOpType.add,
                        )
                    # outT[t, o, ch*128 : ch*128+nrows] = stage[o, t*128 : ...]
                    st = stage[:]
                    out_ap = bass.AP(
                        outT,
                        ch * 128,
                        [[NFIN, OUTF], [OUTF * NFIN, T], [1, nrows]],
                    )
                    st_ap = bass.AP(
                        st.tensor, st.offset, [st.ap[0], [128, T], [1, nrows]]
                    )
                    nc.sync.dma_start(out=out_ap, in_=st_ap)

    nc.compile()
    return nc


_NC_CACHE = {}


def _get_nc(ntiles):
    if ntiles not in _NC_CACHE:
        _NC_CACHE[ntiles] = _build(ntiles)
    return _NC_CACHE[ntiles]


# ================================================================ entry point
def kernel(feat, ef, W, b, src, dst):
    from concourse.bass_utils import run_bass_kernel_spmd

    in_maps, ntiles = _prep(feat, ef, W, b, src, dst)
    nc = _get_nc(ntiles)
    res = run_bass_kernel_spmd(nc, in_maps, core_ids=list(range(CORES)))
    out = np.zeros((N, T, OUTF), np.float32)
    for c in range(CORES):
        o = np.asarray(res.results[c]["outT"], np.float32).reshape(T, OUTF, NFIN)
        if c < 4:
            nbase = c * SPC
        else:
            nbase = (c - 4) * SPC + NFIN
        out[nbase : nbase + NFIN] = o.transpose(2, 0, 1)
    return out
